# revision 14
# baseline (speedup 1.0000x reference)
"""Multi-head self-attention (B=2, S=2048, D=1024, H=16) on 8 Trainium2 NeuronCores.

Sharding: batch x head-group. Core c = b*4 + g handles batch b and heads 4g..4g+3
(Megatron-style TP: Wq/Wk/Wv column-sharded, Wo row-sharded; partial outputs
summed on the host).

v2 layout (T-layout: sequence on the free dim everywhere), bf16 dataflow:
  inputs (host, bf16): xt [1024,2048]=x[b].T; wq/wk/wv [1024,256] (scale-folded,
  transposed); wo [256,1024]
  QT/KT = (w.T @ xt) [256,2048]  K=128 matmuls accumulated in PSUM, evac bf16
  V     = (xt.T @ wv) [2048,260] natural + ones column per head (denominators)
  scoresT[k,q] = KT_h.T @ QT_h   K=64 row-tiled pairs (T0/T8 overlap on HW)
  exp on ACT (scale=1/8) -> full-probs SBUF tile per (hp,n), bf16
  ctxT_aug[65,q] = [V_h|1].T @ expT   K=128, PSUM-accumulated over 16 kc
  norm: recip_approx_fast(denom row) -> gpsimd partition_broadcast -> DVE mult
  outT_partial = wo.T @ ctxT  K=128, fp32 out

Phase 2 software pipeline: per (hp,n) emit 11 score+exp chunks (PSUM: 2x3-bank
ping-pong) interleaved with the PREVIOUS (hp,n)'s 16 ctx units + norm, so the
PE never idles long enough for HAM to re-throttle (the v1 failure mode: phase 2
ran entirely at 1.2 GHz with a 2.1us EXP stall per iteration).
"""
import sys

sys.path.insert(0, "/opt/trn_rl_repo")

import numpy as np

import concourse.bass as bass
import concourse.tile as tile
from concourse import bacc, library_config, mybir
from concourse.bass_utils import run_bass_kernel_spmd

F32 = mybir.dt.float32
BF16 = mybir.dt.bfloat16

S = 2048          # sequence length per batch
D = 1024          # embedding dim
HG = 4            # heads per core
HD = 64           # head dim
GC = HG * HD      # group cols = 256
P = 128
NQ = 4            # q chunks of 512
QW = 512          # q chunk width
NKC = 16          # key-position chunks of 128
KO = 8            # contraction chunks of 128 over D
VW = HD + 1       # V columns per head incl. ones column

# phase-2 chunking: 32 j-units (16 kc x 2 heads) per (hp,n), in chunks of 3
CHUNKS = [3, 3, 3, 3, 3, 3, 3, 3, 3, 3, 2]   # sums to 32
assert sum(CHUNKS) == 2 * NKC

_NC_CACHE = {}
DEBUG_DUMPS = False


def _build():
    if "nc" in _NC_CACHE:
        return _NC_CACHE["nc"]
    nc = bacc.Bacc(trn_type="TRN2", target_bir_lowering=False, debug=False)
    xt_d = nc.dram_tensor("xt", [D, S], BF16, kind="ExternalInput")
    wq_d = nc.dram_tensor("wq", [D, GC], BF16, kind="ExternalInput")
    wk_d = nc.dram_tensor("wk", [D, GC], BF16, kind="ExternalInput")
    wv_d = nc.dram_tensor("wv", [D, GC], BF16, kind="ExternalInput")
    wo_d = nc.dram_tensor("wo", [GC, D], BF16, kind="ExternalInput")
    out_d = nc.dram_tensor("out_t", [D, S], BF16, kind="ExternalOutput")
    dbg = None
    if DEBUG_DUMPS:
        dbg = {
            "dbg_qt": nc.dram_tensor("dbg_qt", [P, 2, S], BF16, kind="ExternalOutput"),
            "dbg_kt": nc.dram_tensor("dbg_kt", [P, 2, S], BF16, kind="ExternalOutput"),
            "dbg_va": nc.dram_tensor("dbg_va", [P, NKC, HG * VW], BF16,
                                     kind="ExternalOutput"),
            "dbg_ex": nc.dram_tensor("dbg_ex", [P, 2 * NKC, QW], BF16,
                                     kind="ExternalOutput"),
            "dbg_cp": nc.dram_tensor("dbg_cp", [P, 2, QW], F32, kind="ExternalOutput"),
            "dbg_rd": nc.dram_tensor("dbg_rd", [1, 2, QW], F32, kind="ExternalOutput"),
            "dbg_bc": nc.dram_tensor("dbg_bc", [64, 2, QW], F32, kind="ExternalOutput"),
            "dbg_ct": nc.dram_tensor("dbg_ct", [P, 2, S], BF16, kind="ExternalOutput"),
        }
    with tile.TileContext(nc) as tc:
        _emit(nc, tc, xt_d, wq_d, wk_d, wv_d, wo_d, out_d, dbg)
    nc.compile()
    _NC_CACHE["nc"] = nc
    return nc


def _emit(nc, tc, xt_d, wq_d, wk_d, wv_d, wo_d, out_d, dbg=None):
    nc.gpsimd.load_library(library_config.attn)

    with tc.tile_pool(name="big", bufs=1) as big:
        # ---- persistent SBUF tensors ----
        wo_sb = big.tile([P, 2, D], BF16)
        qt = big.tile([P, 2, S], BF16)     # head h at parts (h%2)*64, chunk h//2
        kt = big.tile([P, 2, S], BF16)
        va = big.tile([P, NKC, HG * VW], BF16)
        ct = big.tile([P, 2, S], BF16)
        xs = big.tile([P, KO, S], BF16)
        wq = big.tile([P, KO, GC], BF16)
        wk = big.tile([P, KO, GC], BF16)
        wv = big.tile([P, KO, GC], BF16)

        # DMA order: wk first (KT is emitted first), then xs chunks, then rest
        nc.sync.dma_start(wk[:], wk_d.rearrange("(ko p) m -> p ko m", p=P))
        for ko in range(KO):
            nc.sync.dma_start(xs[:, ko, :], xt_d[ko * P:(ko + 1) * P, :])
        nc.sync.dma_start(wq[:], wq_d.rearrange("(ko p) m -> p ko m", p=P))
        nc.sync.dma_start(wv[:], wv_d.rearrange("(ko p) m -> p ko m", p=P))
        nc.sync.dma_start(wo_sb[:], wo_d.rearrange("(c p) e -> p c e", p=P))

        # ones columns of V_aug (col HD of each VW-wide head block), bf16 1.0
        va_h = va[:].rearrange("p s (h c) -> p s h c", c=VW)
        for h in range(HG):
            nc.vector.memset(
                va_h[:, :, h, HD:HD + 1].bitcast(mybir.dt.uint16), 0x3F80)

        # ================= phase 1: projections (K=128 matmuls) ============
        with tc.tile_pool(name="ph1", bufs=3, space="PSUM") as ph1:
            def proj_tile(w_sb, m, n, dst):
                """dst[:, m, n*QW:(n+1)*QW] = (w[:, m-chunk]).T @ x.T chunk."""
                ps = ph1.tile([P, QW], F32, tag="p1")
                for ko in range(KO):
                    nc.tensor.matmul(
                        ps[:], w_sb[:, ko, m * P:(m + 1) * P],
                        xs[:, ko, n * QW:(n + 1) * QW],
                        start=(ko == 0), stop=(ko == KO - 1))
                nc.vector.tensor_copy(dst[:, m, n * QW:(n + 1) * QW], ps[:])

            def v_tile(sc):
                """va[:, sc, h, 0:HD] = x.T chunk.T @ wv (natural V layout)."""
                ps = ph1.tile([P, QW], F32, tag="p1")
                for ko in range(KO):
                    nc.tensor.matmul(
                        ps[:, :GC], xs[:, ko, sc * P:(sc + 1) * P],
                        wv[:, ko, :],
                        start=(ko == 0), stop=(ko == KO - 1))
                nc.vector.tensor_copy(
                    va_h[:, sc, :, 0:HD],
                    ps[:, :GC].rearrange("p (h c) -> p h c", c=HD))

            for n in range(NQ):
                proj_tile(wk, 0, n, kt)
            for n in range(NQ):
                proj_tile(wq, 0, n, qt)
            for sc in range(NKC):
                v_tile(sc)
            for n in range(NQ):
                proj_tile(wk, 1, n, kt)
            for n in range(NQ):
                proj_tile(wq, 1, n, qt)

        # ================= phase 2: attention =============================
        # per (hp,n): 11 score+exp chunks into a full-probs sbuf tile;
        # ctx/norm of the previous (hp,n) interleaved into the PE stream.
        HPN = [(hp, n) for hp in range(2) for n in range(NQ)]

        # PSUM budget (8 banks): sp 2x[128,3,512]=6 banks + cpp 2 tags x 1 bank.
        # Phase-3 psum tiles SHARE the cpp tags so rotation order == program
        # order (a separate pool would need a 9th bank / could deadlock).
        with tc.tile_pool(name="sp", bufs=2, space="PSUM") as spp, \
             tc.tile_pool(name="cpp", bufs=1, space="PSUM") as cpp, \
             tc.tile_pool(name="exp", bufs=2) as exp_pool, \
             tc.tile_pool(name="nrm", bufs=2) as nrm, \
             tc.tile_pool(name="ot", bufs=2) as otp:

            ex_tiles = {}   # i -> ex_full tile

            def emit_scores_chunk(hp, n, c, ex_full):
                """Chunk c: CHUNKS[c] score matmuls + one exp into ex_full."""
                j0 = sum(CHUNKS[:c])
                w = CHUNKS[c]
                sp = spp.tile([P, 3, QW], F32, tag="sp")
                for jj in range(w):
                    j = j0 + jj
                    kc, e = j // 2, j % 2
                    lo = e * 64
                    nc.tensor.matmul(
                        sp[:, jj, :],
                        kt[lo:lo + 64, hp, kc * P:(kc + 1) * P],
                        qt[lo:lo + 64, hp, n * QW:(n + 1) * QW],
                        start=True, stop=True)
                nc.scalar.activation(
                    ex_full[:, j0:j0 + w, :].rearrange("p a b -> p (a b)"),
                    sp[:, 0:w, :].rearrange("p a b -> p (a b)"),
                    mybir.ActivationFunctionType.Exp,
                    scale=0.125)

            def emit_ctx_unit(hp, n, kc, ex_full, cps):
                """ctx accumulation for one kc (both heads of the pair)."""
                for e in range(2):
                    h = 2 * hp + e
                    j = kc * 2 + e
                    nc.tensor.matmul(
                        cps[e][0:VW, :],
                        va[0:P, kc, h * VW:(h + 1) * VW],
                        ex_full[:, j, :],
                        start=(kc == 0), stop=(kc == NKC - 1))

            def emit_norm(hp, n, cps):
                """ct slice = cps[0:HD] * broadcast(1/cps[HD])."""
                for e in range(2):
                    dn = nrm.tile([1, QW], F32, tag=f"dn{e}")
                    rd = nrm.tile([1, QW], F32, tag=f"rd{e}")
                    bc = nrm.tile([64, QW], F32, tag=f"bc{e}")
                    nc.vector.tensor_copy(dn[:], cps[e][HD:VW, :])
                    nc.vector.reciprocal_approx_fast(out=rd[:], in_=dn[:])
                    nc.gpsimd.partition_broadcast(bc[:], rd[:], channels=64)
                    nc.vector.tensor_tensor(
                        ct[e * 64:(e + 1) * 64, hp, n * QW:(n + 1) * QW],
                        cps[e][0:HD, :], bc[:], mybir.AluOpType.mult)
                    if dbg is not None and hp == 0 and n == 0:
                        cpc = nrm.tile([P, QW], F32, tag=f"cpd{e}")
                        nc.vector.tensor_copy(cpc[:], cps[e][:])
                        nc.sync.dma_start(dbg["dbg_cp"][:, e, :], cpc[:])
                        nc.sync.dma_start(dbg["dbg_rd"][:, e, :], rd[:])
                        nc.sync.dma_start(dbg["dbg_bc"][:, e, :], bc[:])

            def emit_ph3_tile(n, m):
                """one output-projection m-tile of q-chunk n (cpp banks)."""
                ps = cpp.tile([P, QW], F32, tag=f"cp{m % 2}",
                              name=f"p3_{n}_{m}")
                for c in range(2):
                    nc.tensor.matmul(
                        ps[:], wo_sb[:, c, m * P:(m + 1) * P],
                        ct[:, c, n * QW:(n + 1) * QW],
                        start=(c == 0), stop=(c == 1))
                ot = otp.tile([P, QW], BF16, tag="ot")
                nc.vector.tensor_copy(ot[:], ps[:])
                nc.sync.dma_start(
                    out_d[m * P:(m + 1) * P, n * QW:(n + 1) * QW], ot[:])

            def emit_ph3_n_sp(n):
                """output projection for q-chunk n through the (freed) score
                psum banks, 3 m-tiles at a time — used for the final drain."""
                for m0 in range(0, KO, 3):
                    mw = min(3, KO - m0)
                    ps = spp.tile([P, 3, QW], F32, tag="sp")
                    for mm in range(mw):
                        m = m0 + mm
                        for c in range(2):
                            nc.tensor.matmul(
                                ps[:, mm, :], wo_sb[:, c, m * P:(m + 1) * P],
                                ct[:, c, n * QW:(n + 1) * QW],
                                start=(c == 0), stop=(c == 1))
                    ot = otp.tile([P, 3, QW], BF16, tag="ot3")
                    nc.vector.tensor_copy(ot[:, 0:mw, :], ps[:, 0:mw, :])
                    for mm in range(mw):
                        nc.sync.dma_start(
                            out_d[(m0 + mm) * P:(m0 + mm + 1) * P,
                                  n * QW:(n + 1) * QW], ot[:, mm, :])

            # ---- work queue: deferred PE work pumped between score chunks
            # (FIFO order keeps cpp pool rotation == program order) ----
            from collections import deque
            pending = deque()   # (weight, closure)

            def pump(budget):
                while pending and budget > 0:
                    w, fn = pending.popleft()
                    fn()
                    budget -= w
                return budget

            cps_holder = {}

            def push_iter_work(i):
                """queue ctx units, norm, and phase-3 tiles for iteration i."""
                php, pn = HPN[i]

                def mk_ctx(kc):
                    def fn():
                        if cps_holder.get(i) is None:
                            cps_holder[i] = [
                                cpp.tile([P, QW], F32, tag=f"cp{e}",
                                         name=f"cp{e}_{i}")
                                for e in range(2)]
                        emit_ctx_unit(php, pn, kc, ex_tiles[i], cps_holder[i])
                    return fn

                def fn_norm():
                    emit_norm(php, pn, cps_holder[i])
                    del ex_tiles[i]
                    del cps_holder[i]

                for kc in range(NKC):
                    pending.append((1.0, mk_ctx(kc)))
                pending.append((0.3, fn_norm))
                if php == 1 and pn < 3:
                    for m in range(KO):
                        pending.append((1.5, lambda n=pn, m=m: emit_ph3_tile(n, m)))

            for i, (hp, n) in enumerate(HPN):
                ex_tiles[i] = exp_pool.tile([P, 2 * NKC, QW], BF16, tag="ex",
                                            name=f"ex_{i}")
                last = i == len(HPN) - 1

                def mk_ctx_self(kc, i=i, hp=hp, n=n):
                    def fn():
                        if cps_holder.get(i) is None:
                            cps_holder[i] = [
                                cpp.tile([P, QW], F32, tag=f"cp{e}",
                                         name=f"cp{e}_{i}")
                                for e in range(2)]
                        emit_ctx_unit(hp, n, kc, ex_tiles[i], cps_holder[i])
                    return fn

                kc_self = 0
                for c in range(len(CHUNKS)):
                    emit_scores_chunk(hp, n, c, ex_tiles[i])
                    if last:
                        # own ctx units become eligible as their exps are
                        # emitted; queue FIFO keeps cpp alloc order correct
                        safe = (3 * (c + 1) - 1) // 2
                        while kc_self < min(safe, NKC):
                            pending.append((1.0, mk_ctx_self(kc_self)))
                            kc_self += 1
                    pump(5.0 if last else 2.5)
                if dbg is not None and i == 0:
                    nc.sync.dma_start(dbg["dbg_ex"][:], ex_tiles[i][:])
                if not last:
                    push_iter_work(i)
                else:
                    php, pn = HPN[i]

                    def fn_norm_last():
                        emit_norm(php, pn, cps_holder[i])
                    pending.append((0.3, fn_norm_last))

            # drain everything left, then the last output projection
            pump(1e9)
            emit_ph3_n_sp(3)

            if dbg is not None:
                nc.sync.dma_start(dbg["dbg_qt"][:], qt[:])
                nc.sync.dma_start(dbg["dbg_kt"][:], kt[:])
                nc.sync.dma_start(dbg["dbg_va"][:], va[:])
                nc.sync.dma_start(dbg["dbg_ct"][:], ct[:])


def _in_maps(x, wq_f, wk_f, wv_f, wo_f):
    maps = []
    for core in range(8):
        b, g = core // 4, core % 4
        cols = slice(g * GC, (g + 1) * GC)
        maps.append({
            "xt": np.ascontiguousarray(x[b].T),
            "wq": np.ascontiguousarray(wq_f[:, cols]),
            "wk": np.ascontiguousarray(wk_f[:, cols]),
            "wv": np.ascontiguousarray(wv_f[:, cols]),
            "wo": np.ascontiguousarray(wo_f[cols, :]),
        })
    return maps


def _to_bf16(a):
    """fp32 -> bf16 (round-to-nearest-even) stored as uint16 view for ml_dtypes-
    free numpy; bass accepts uint16-backed arrays for bf16 dram tensors?  Use
    ml_dtypes if available, else manual rounding."""
    try:
        import ml_dtypes
        return a.astype(ml_dtypes.bfloat16)
    except ImportError:
        a32 = np.asarray(a, dtype=np.float32)
        u = a32.view(np.uint32)
        rounded = ((u + 0x7FFF + ((u >> 16) & 1)) >> 16).astype(np.uint16)
        return rounded


def _prep(x, Wq, Wk, Wv, Wo, q_scale, k_scale, v_scale, o_scale):
    x = np.asarray(x, dtype=np.float32)
    wq_f = (np.asarray(Wq).T * np.asarray(q_scale).reshape(1, -1)).astype(np.float32)
    wk_f = (np.asarray(Wk).T * np.asarray(k_scale).reshape(1, -1)).astype(np.float32)
    wv_f = (np.asarray(Wv).T * np.asarray(v_scale).reshape(1, -1)).astype(np.float32)
    wo_f = (np.asarray(Wo).T * np.asarray(o_scale).reshape(1, -1)).astype(np.float32)
    maps = _in_maps(x, wq_f, wk_f, wv_f, wo_f)
    maps = [{k: _to_bf16(v) for k, v in m.items()} for m in maps]
    return x, maps


def run_traced(x, Wq, Wk, Wv, Wo, q_scale, k_scale, v_scale, o_scale):
    x, maps = _prep(x, Wq, Wk, Wv, Wo, q_scale, k_scale, v_scale, o_scale)
    nc = _build()
    res = run_bass_kernel_spmd(nc, maps, core_ids=list(range(8)), trace=True)
    out = np.zeros((x.shape[0], S, D), dtype=np.float32)
    for core in range(8):
        out[core // 4] += np.asarray(res.results[core]["out_t"],
                                     dtype=np.float32).T
    trace_path = None
    if res.instructions_and_trace is not None:
        trace_path = res.instructions_and_trace[1]
    return out, res.exec_time_ns, trace_path


def kernel(x, Wq, Wk, Wv, Wo, q_scale, k_scale, v_scale, o_scale):
    x, maps = _prep(x, Wq, Wk, Wv, Wo, q_scale, k_scale, v_scale, o_scale)
    nc = _build()
    res = run_bass_kernel_spmd(nc, maps, core_ids=list(range(8)))
    out = np.zeros((x.shape[0], S, D), dtype=np.float32)
    for core in range(8):
        out[core // 4] += np.asarray(res.results[core]["out_t"],
                                     dtype=np.float32).T
    return out


# revision 18
# speedup vs baseline: 1.1651x; 1.1651x over previous
"""Multi-head self-attention (B=2, S=2048, D=1024, H=16) on 8 Trainium2 NeuronCores.

Sharding: batch x head-group. Core c = b*4 + g handles batch b and heads 4g..4g+3
(Megatron-style TP: Wq/Wk/Wv column-sharded, Wo row-sharded; partial outputs
summed on the host).

v2 layout (T-layout: sequence on the free dim everywhere), bf16 dataflow:
  inputs (host, bf16): xt [1024,2048]=x[b].T; wq/wk/wv [1024,256] (scale-folded,
  transposed); wo [256,1024]
  QT/KT = (w.T @ xt) [256,2048]  K=128 matmuls accumulated in PSUM, evac bf16
  V     = (xt.T @ wv) [2048,260] natural + ones column per head (denominators)
  scoresT[k,q] = KT_h.T @ QT_h   K=64 row-tiled pairs (T0/T8 overlap on HW)
  exp on ACT (scale=1/8) -> full-probs SBUF tile per (hp,n), bf16
  ctxT_aug[65,q] = [V_h|1].T @ expT   K=128, PSUM-accumulated over 16 kc
  norm: recip_approx_fast(denom row) -> gpsimd partition_broadcast -> DVE mult
  outT_partial = wo.T @ ctxT  K=128, fp32 out

Phase 2 software pipeline: per (hp,n) emit 11 score+exp chunks (PSUM: 2x3-bank
ping-pong) interleaved with the PREVIOUS (hp,n)'s 16 ctx units + norm, so the
PE never idles long enough for HAM to re-throttle (the v1 failure mode: phase 2
ran entirely at 1.2 GHz with a 2.1us EXP stall per iteration).
"""
import sys

sys.path.insert(0, "/opt/trn_rl_repo")

import numpy as np

import concourse.bass as bass
import concourse.tile as tile
from concourse import bacc, library_config, mybir
from concourse.bass_utils import run_bass_kernel_spmd

F32 = mybir.dt.float32
BF16 = mybir.dt.bfloat16

S = 2048          # sequence length per batch
D = 1024          # embedding dim
HG = 4            # heads per core
HD = 64           # head dim
GC = HG * HD      # group cols = 256
P = 128
NQ = 4            # q chunks of 512
QW = 512          # q chunk width
NKC = 16          # key-position chunks of 128
KO = 8            # contraction chunks of 128 over D
VW = HD + 1       # V columns per head incl. ones column

# phase-2 chunking: 32 j-units (16 kc x 2 heads) per (hp,n), in chunks of 3
CHUNKS = [3, 3, 3, 3, 3, 3, 3, 3, 3, 3, 2]   # sums to 32
assert sum(CHUNKS) == 2 * NKC

_NC_CACHE = {}
DEBUG_DUMPS = False


def _build():
    if "nc" in _NC_CACHE:
        return _NC_CACHE["nc"]
    nc = bacc.Bacc(trn_type="TRN2", target_bir_lowering=False, debug=False)
    xt_d = nc.dram_tensor("xt", [D, S], BF16, kind="ExternalInput")
    wq_d = nc.dram_tensor("wq", [D, GC], BF16, kind="ExternalInput")
    wk_d = nc.dram_tensor("wk", [D, GC], BF16, kind="ExternalInput")
    wv_d = nc.dram_tensor("wv", [D, GC], BF16, kind="ExternalInput")
    wo_d = nc.dram_tensor("wo", [GC, D], BF16, kind="ExternalInput")
    out_d = nc.dram_tensor("out_t", [D, S], BF16, kind="ExternalOutput")
    dbg = None
    if DEBUG_DUMPS:
        dbg = {
            "dbg_qt": nc.dram_tensor("dbg_qt", [P, 2, S], BF16, kind="ExternalOutput"),
            "dbg_kt": nc.dram_tensor("dbg_kt", [P, 2, S], BF16, kind="ExternalOutput"),
            "dbg_va": nc.dram_tensor("dbg_va", [P, NKC, HG * VW], BF16,
                                     kind="ExternalOutput"),
            "dbg_ex": nc.dram_tensor("dbg_ex", [P, 2 * NKC, QW], BF16,
                                     kind="ExternalOutput"),
            "dbg_cp": nc.dram_tensor("dbg_cp", [P, 2, QW], F32, kind="ExternalOutput"),
            "dbg_rd": nc.dram_tensor("dbg_rd", [1, 2, QW], F32, kind="ExternalOutput"),
            "dbg_bc": nc.dram_tensor("dbg_bc", [64, 2, QW], F32, kind="ExternalOutput"),
            "dbg_ct": nc.dram_tensor("dbg_ct", [P, 2, S], BF16, kind="ExternalOutput"),
        }
    with tile.TileContext(nc) as tc:
        _emit(nc, tc, xt_d, wq_d, wk_d, wv_d, wo_d, out_d, dbg)
    nc.compile()
    _NC_CACHE["nc"] = nc
    return nc


def _emit(nc, tc, xt_d, wq_d, wk_d, wv_d, wo_d, out_d, dbg=None):
    nc.gpsimd.load_library(library_config.attn)

    with tc.tile_pool(name="big", bufs=1) as big:
        # ---- persistent SBUF tensors ----
        wo_sb = big.tile([P, 2, D], BF16)
        qt = big.tile([P, 2, S], BF16)     # head h at parts (h%2)*64, chunk h//2
        kt = big.tile([P, 2, S], BF16)
        va = big.tile([P, NKC, HG * VW], BF16)
        ct = big.tile([P, 2, S], BF16)
        xs = big.tile([P, KO, S], BF16)
        wq = big.tile([P, KO, GC], BF16)
        wk = big.tile([P, KO, GC], BF16)
        wv = big.tile([P, KO, GC], BF16)

        # DMA order: wk first (KT is emitted first), then xs chunks, then rest
        nc.sync.dma_start(wk[:], wk_d.rearrange("(ko p) m -> p ko m", p=P))
        for ko in range(KO):
            nc.sync.dma_start(xs[:, ko, :], xt_d[ko * P:(ko + 1) * P, :])
        nc.sync.dma_start(wq[:], wq_d.rearrange("(ko p) m -> p ko m", p=P))
        nc.sync.dma_start(wv[:], wv_d.rearrange("(ko p) m -> p ko m", p=P))
        nc.sync.dma_start(wo_sb[:], wo_d.rearrange("(c p) e -> p c e", p=P))

        # ones columns of V_aug (col HD of each VW-wide head block), bf16 1.0
        va_h = va[:].rearrange("p s (h c) -> p s h c", c=VW)
        for h in range(HG):
            nc.vector.memset(
                va_h[:, :, h, HD:HD + 1].bitcast(mybir.dt.uint16), 0x3F80)

        # ================= phase 1: projections (K=128 matmuls) ============
        with tc.tile_pool(name="ph1", bufs=3, space="PSUM") as ph1:
            def proj_tile(w_sb, m, n, dst):
                """dst[:, m, n*QW:(n+1)*QW] = (w[:, m-chunk]).T @ x.T chunk."""
                ps = ph1.tile([P, QW], F32, tag="p1")
                for ko in range(KO):
                    nc.tensor.matmul(
                        ps[:], w_sb[:, ko, m * P:(m + 1) * P],
                        xs[:, ko, n * QW:(n + 1) * QW],
                        start=(ko == 0), stop=(ko == KO - 1))
                nc.vector.tensor_copy(dst[:, m, n * QW:(n + 1) * QW], ps[:])

            def v_tile(sc):
                """va[:, sc, h, 0:HD] = x.T chunk.T @ wv (natural V layout)."""
                ps = ph1.tile([P, QW], F32, tag="p1")
                for ko in range(KO):
                    nc.tensor.matmul(
                        ps[:, :GC], xs[:, ko, sc * P:(sc + 1) * P],
                        wv[:, ko, :],
                        start=(ko == 0), stop=(ko == KO - 1))
                nc.vector.tensor_copy(
                    va_h[:, sc, :, 0:HD],
                    ps[:, :GC].rearrange("p (h c) -> p h c", c=HD))

            for n in range(NQ):
                proj_tile(wk, 0, n, kt)
            for n in range(NQ):
                proj_tile(wq, 0, n, qt)
            for sc in range(NKC):
                v_tile(sc)
            for n in range(NQ):
                proj_tile(wk, 1, n, kt)
            for n in range(NQ):
                proj_tile(wq, 1, n, qt)

        # ================= phase 2: attention =============================
        # per (hp,n): 11 score+exp chunks into a full-probs sbuf tile;
        # ctx/norm of the previous (hp,n) interleaved into the PE stream.
        HPN = [(hp, n) for hp in range(2) for n in range(NQ)]

        # PSUM budget (8 banks): sp 2x[128,3,512]=6 banks + cpp 2 tags x 1 bank.
        # Phase-3 psum tiles SHARE the cpp tags so rotation order == program
        # order (a separate pool would need a 9th bank / could deadlock).
        with tc.tile_pool(name="sp", bufs=2, space="PSUM") as spp, \
             tc.tile_pool(name="cpp", bufs=1, space="PSUM") as cpp, \
             tc.tile_pool(name="exp", bufs=2) as exp_pool, \
             tc.tile_pool(name="nrm", bufs=2) as nrm, \
             tc.tile_pool(name="ot", bufs=2) as otp:

            ex_tiles = {}   # i -> ex_full tile

            def emit_scores_chunk(hp, n, c, ex_full):
                """Chunk c: CHUNKS[c] score matmuls + one exp into ex_full."""
                j0 = sum(CHUNKS[:c])
                w = CHUNKS[c]
                sp = spp.tile([P, 3, QW], F32, tag="sp")
                for jj in range(w):
                    j = j0 + jj
                    kc, e = j // 2, j % 2
                    lo = e * 64
                    nc.tensor.matmul(
                        sp[:, jj, :],
                        kt[lo:lo + 64, hp, kc * P:(kc + 1) * P],
                        qt[lo:lo + 64, hp, n * QW:(n + 1) * QW],
                        start=True, stop=True)
                nc.scalar.activation(
                    ex_full[:, j0:j0 + w, :].rearrange("p a b -> p (a b)"),
                    sp[:, 0:w, :].rearrange("p a b -> p (a b)"),
                    mybir.ActivationFunctionType.Exp,
                    scale=0.125)

            def emit_ctx_unit(hp, n, kc, ex_full, cps):
                """ctx accumulation for one kc (both heads of the pair)."""
                for e in range(2):
                    h = 2 * hp + e
                    j = kc * 2 + e
                    nc.tensor.matmul(
                        cps[e][0:VW, :],
                        va[0:P, kc, h * VW:(h + 1) * VW],
                        ex_full[:, j, :],
                        start=(kc == 0), stop=(kc == NKC - 1))

            def emit_norm(hp, n, cps):
                """ct slice = cps[0:HD] * broadcast(1/cps[HD])."""
                for e in range(2):
                    dn = nrm.tile([1, QW], F32, tag=f"dn{e}")
                    rd = nrm.tile([1, QW], F32, tag=f"rd{e}")
                    bc = nrm.tile([64, QW], F32, tag=f"bc{e}")
                    nc.vector.tensor_copy(dn[:], cps[e][HD:VW, :])
                    nc.vector.reciprocal_approx_fast(out=rd[:], in_=dn[:])
                    nc.gpsimd.partition_broadcast(bc[:], rd[:], channels=64)
                    nc.vector.tensor_tensor(
                        ct[e * 64:(e + 1) * 64, hp, n * QW:(n + 1) * QW],
                        cps[e][0:HD, :], bc[:], mybir.AluOpType.mult)
                    if dbg is not None and hp == 0 and n == 0:
                        cpc = nrm.tile([P, QW], F32, tag=f"cpd{e}")
                        nc.vector.tensor_copy(cpc[:], cps[e][:])
                        nc.sync.dma_start(dbg["dbg_cp"][:, e, :], cpc[:])
                        nc.sync.dma_start(dbg["dbg_rd"][:, e, :], rd[:])
                        nc.sync.dma_start(dbg["dbg_bc"][:, e, :], bc[:])

            def emit_ph3_n_sp(n):
                """output projection for q-chunk n through the (freed) score
                psum banks, 3 m-tiles at a time — used for the final drain."""
                for m0 in range(0, KO, 3):
                    mw = min(3, KO - m0)
                    ps = spp.tile([P, 3, QW], F32, tag="sp")
                    for mm in range(mw):
                        m = m0 + mm
                        for c in range(2):
                            nc.tensor.matmul(
                                ps[:, mm, :], wo_sb[:, c, m * P:(m + 1) * P],
                                ct[:, c, n * QW:(n + 1) * QW],
                                start=(c == 0), stop=(c == 1))
                    ot = otp.tile([P, 3, QW], BF16, tag="ot3")
                    nc.vector.tensor_copy(ot[:, 0:mw, :], ps[:, 0:mw, :])
                    for mm in range(mw):
                        nc.sync.dma_start(
                            out_d[(m0 + mm) * P:(m0 + mm + 1) * P,
                                  n * QW:(n + 1) * QW], ot[:, mm, :])

            # ---- work queue: deferred PE work pumped between score chunks
            # (FIFO order keeps cpp pool rotation == program order) ----
            from collections import deque
            pending = deque()   # (weight, closure)

            def pump(budget):
                while pending and budget > 0:
                    w, fn = pending.popleft()
                    fn()
                    budget -= w
                return budget

            cps_holder = {}

            def push_iter_work(i):
                """queue ctx units, norm, and phase-3 tiles for iteration i."""
                php, pn = HPN[i]

                def mk_ctx(kc):
                    def fn():
                        if cps_holder.get(i) is None:
                            cps_holder[i] = [
                                cpp.tile([P, QW], F32, tag=f"cp{e}",
                                         name=f"cp{e}_{i}")
                                for e in range(2)]
                        emit_ctx_unit(php, pn, kc, ex_tiles[i], cps_holder[i])
                    return fn

                def fn_norm():
                    emit_norm(php, pn, cps_holder[i])
                    del ex_tiles[i]
                    del cps_holder[i]

                for kc in range(NKC):
                    pending.append((1.0, mk_ctx(kc)))
                pending.append((0.3, fn_norm))

            for i, (hp, n) in enumerate(HPN):
                ex_tiles[i] = exp_pool.tile([P, 2 * NKC, QW], BF16, tag="ex",
                                            name=f"ex_{i}")
                last = i == len(HPN) - 1

                def mk_ctx_self(kc, i=i, hp=hp, n=n):
                    def fn():
                        if cps_holder.get(i) is None:
                            cps_holder[i] = [
                                cpp.tile([P, QW], F32, tag=f"cp{e}",
                                         name=f"cp{e}_{i}")
                                for e in range(2)]
                        emit_ctx_unit(hp, n, kc, ex_tiles[i], cps_holder[i])
                    return fn

                kc_self = 0
                for c in range(len(CHUNKS)):
                    emit_scores_chunk(hp, n, c, ex_tiles[i])
                    if last:
                        # own ctx units become eligible as their exps are
                        # emitted; queue FIFO keeps cpp alloc order correct
                        safe = (3 * (c + 1) - 1) // 2
                        while kc_self < min(safe, NKC):
                            pending.append((1.0, mk_ctx_self(kc_self)))
                            kc_self += 1
                    pump(5.0 if last else 1.6)
                if dbg is not None and i == 0:
                    nc.sync.dma_start(dbg["dbg_ex"][:], ex_tiles[i][:])
                if not last:
                    push_iter_work(i)
                else:
                    php, pn = HPN[i]

                    def fn_norm_last():
                        emit_norm(php, pn, cps_holder[i])
                    pending.append((0.3, fn_norm_last))

            # drain everything left, then the whole output projection as one
            # dense PE burst through the freed score-psum banks
            pump(1e9)
            for n in range(NQ):
                emit_ph3_n_sp(n)

            if dbg is not None:
                nc.sync.dma_start(dbg["dbg_qt"][:], qt[:])
                nc.sync.dma_start(dbg["dbg_kt"][:], kt[:])
                nc.sync.dma_start(dbg["dbg_va"][:], va[:])
                nc.sync.dma_start(dbg["dbg_ct"][:], ct[:])


def _in_maps(x, wq_f, wk_f, wv_f, wo_f):
    maps = []
    for core in range(8):
        b, g = core // 4, core % 4
        cols = slice(g * GC, (g + 1) * GC)
        maps.append({
            "xt": np.ascontiguousarray(x[b].T),
            "wq": np.ascontiguousarray(wq_f[:, cols]),
            "wk": np.ascontiguousarray(wk_f[:, cols]),
            "wv": np.ascontiguousarray(wv_f[:, cols]),
            "wo": np.ascontiguousarray(wo_f[cols, :]),
        })
    return maps


def _to_bf16(a):
    """fp32 -> bf16 (round-to-nearest-even) stored as uint16 view for ml_dtypes-
    free numpy; bass accepts uint16-backed arrays for bf16 dram tensors?  Use
    ml_dtypes if available, else manual rounding."""
    try:
        import ml_dtypes
        return a.astype(ml_dtypes.bfloat16)
    except ImportError:
        a32 = np.asarray(a, dtype=np.float32)
        u = a32.view(np.uint32)
        rounded = ((u + 0x7FFF + ((u >> 16) & 1)) >> 16).astype(np.uint16)
        return rounded


def _prep(x, Wq, Wk, Wv, Wo, q_scale, k_scale, v_scale, o_scale):
    x = np.asarray(x, dtype=np.float32)
    wq_f = (np.asarray(Wq).T * np.asarray(q_scale).reshape(1, -1)).astype(np.float32)
    wk_f = (np.asarray(Wk).T * np.asarray(k_scale).reshape(1, -1)).astype(np.float32)
    wv_f = (np.asarray(Wv).T * np.asarray(v_scale).reshape(1, -1)).astype(np.float32)
    wo_f = (np.asarray(Wo).T * np.asarray(o_scale).reshape(1, -1)).astype(np.float32)
    maps = _in_maps(x, wq_f, wk_f, wv_f, wo_f)
    maps = [{k: _to_bf16(v) for k, v in m.items()} for m in maps]
    return x, maps


def run_traced(x, Wq, Wk, Wv, Wo, q_scale, k_scale, v_scale, o_scale):
    x, maps = _prep(x, Wq, Wk, Wv, Wo, q_scale, k_scale, v_scale, o_scale)
    nc = _build()
    res = run_bass_kernel_spmd(nc, maps, core_ids=list(range(8)), trace=True)
    out = np.zeros((x.shape[0], S, D), dtype=np.float32)
    for core in range(8):
        out[core // 4] += np.asarray(res.results[core]["out_t"],
                                     dtype=np.float32).T
    trace_path = None
    if res.instructions_and_trace is not None:
        trace_path = res.instructions_and_trace[1]
    return out, res.exec_time_ns, trace_path


def kernel(x, Wq, Wk, Wv, Wo, q_scale, k_scale, v_scale, o_scale):
    x, maps = _prep(x, Wq, Wk, Wv, Wo, q_scale, k_scale, v_scale, o_scale)
    nc = _build()
    res = run_bass_kernel_spmd(nc, maps, core_ids=list(range(8)))
    out = np.zeros((x.shape[0], S, D), dtype=np.float32)
    for core in range(8):
        out[core // 4] += np.asarray(res.results[core]["out_t"],
                                     dtype=np.float32).T
    return out
